# revision 13
# baseline (speedup 1.0000x reference)
"""MoE layer (8 experts, top-2 routing, SwiGLU) on 8 Trainium2 NeuronCores.

Strategy (expert-parallel, capacity-based sparse dispatch):
  Launch 1 (router, data-parallel over tokens): each core computes fp32
    router logits for its 1024-token shard and emits the dense [T,8]
    combine-weight matrix (top-2 softmax weights, exact zeros elsewhere).
  Host: builds per-expert token index lists from the exact zero pattern,
    pads to a fixed capacity, gathers token columns (bf16) per expert.
  Launch 2 (experts, one expert per core): each core runs the SwiGLU MLP
    for its expert over its gathered tokens in bf16 (fp32 accumulate),
    scales by the combine weight, and returns y^T [H, CAP].
  Host: scatter-adds the per-expert outputs into the full [B,S,H] result.
"""

import numpy as np
import ml_dtypes

import concourse.bass as bass
import concourse.mybir as mybir
import concourse.tile as tile
from concourse.bass_utils import run_bass_kernel_spmd
from concourse.vector_clock import ScopedClock

BF16 = mybir.dt.bfloat16
F32 = mybir.dt.float32
AF = mybir.ActivationFunctionType
ALU = mybir.AluOpType
AX = mybir.AxisListType

H = 1024
I = 4096
E = 8
T = 8192
TPC = T // 8          # tokens per core in the router launch
CAP = 2240            # per-expert token capacity (max observed load 2182)
HS = H // 128         # 8 H sub-tiles
IS = I // 128         # 32 I sub-tiles
NP_BF16 = ml_dtypes.bfloat16


def _t_tiles(cap):
    """Split cap into <=512-wide token tiles (PSUM bank = 512 fp32)."""
    tiles, t0 = [], 0
    while t0 < cap:
        tiles.append((t0, min(512, cap - t0)))
        t0 += 512
    return tiles


_MAX_WAITS = 1  # this walrus build rejects multiple sync waits on one instruction


class _TileContext(tile.TileContext):
    """TileContext that hoists excess per-instruction semaphore waits into
    standalone same-engine nops; the walrus build here caps the number of
    sync waits a single instruction may carry."""

    def _add_instruction(self, inst):
        si = getattr(inst, "sync_info", None)
        if (
            si is not None
            and len(si.on_wait) > _MAX_WAITS
            and inst.engine != mybir.EngineType.Unassigned
        ):
            waits = list(si.on_wait)
            hoist, keep = waits[:-_MAX_WAITS], waits[-_MAX_WAITS:]
            for k in range(0, len(hoist), _MAX_WAITS):
                nop = mybir.InstNoOp(
                    name=self.nc.get_next_instruction_name(), ins=[], outs=[]
                )
                nop.engine = inst.engine
                nop.sync_info = mybir.SyncInfo(
                    on_wait=hoist[k : k + _MAX_WAITS], on_update=[]
                )
                super()._add_instruction(nop)
            si.on_wait = keep
        super()._add_instruction(inst)

    def _drain_and_barrier(self, tick_clock, wait_clock):
        nc = self.nc
        probe = nc.sync.nop(nofuse=True)
        wait_clock.add_sem_waits(
            probe.ins, ScopedClock({None: tick_clock.global_clock})
        )
        si = probe.ins.sync_info
        waits = list(si.on_wait) if si is not None else []
        if si is not None:
            si.on_wait = waits[:_MAX_WAITS]
        for k in range(_MAX_WAITS, len(waits), _MAX_WAITS):
            n = nc.sync.nop(nofuse=True)
            n.ins.sync_info = mybir.SyncInfo(
                on_wait=waits[k : k + _MAX_WAITS], on_update=[]
            )
        nc.sync.drain()
        nc.all_engine_barrier()
        popped = nc._tile_sem_poison_stack.pop()
        assert popped is self._sem_poison
        nc.clear_and_free_semaphores(list(self.sems.allocated().values()))
        nc.all_engine_barrier()


def build_router() -> bass.Bass:
    """Per-core: logits = x @ gate_w in fp32, top-2 softmax -> dense [TPC, E]
    combine weights (exact 0 for unselected experts).

    Inputs:  xt [128, HS, TPC] fp32  (xt[p, s, t] = x[t, s*128+p])
             gw [128, HS, E]  fp32  (gw[p, s, e] = gate_w[s*128+p, e])
    Output:  wd [TPC, E] fp32
    """
    nc = bass.Bass()
    xt = nc.dram_tensor("xt", [128, HS, TPC], F32, kind="ExternalInput")
    gw = nc.dram_tensor("gw", [128, HS, E], F32, kind="ExternalInput")
    wd = nc.dram_tensor("wd", [TPC, E], F32, kind="ExternalOutput")

    with _TileContext(nc) as tc:
        with (
            tc.tile_pool(name="const", bufs=1) as const,
            tc.tile_pool(name="work", bufs=8) as work,
            tc.tile_pool(name="psum", bufs=6, space="PSUM") as psum,
        ):
            gw_sb = const.tile([128, HS, E], F32, tag="gw")
            nc.sync.dma_start(out=gw_sb[:], in_=gw[:])

            for tb in range(TPC // 128):
                xt_sb = work.tile([128, HS, 128], F32, tag="xt")
                nc.sync.dma_start(
                    out=xt_sb[:], in_=xt[:, :, tb * 128 : (tb + 1) * 128]
                )
                pl = psum.tile([128, E], F32, tag="pl")
                for s in range(HS):
                    nc.tensor.matmul(
                        pl[:],
                        lhsT=xt_sb[:, s, :],
                        rhs=gw_sb[:, s, :],
                        start=(s == 0),
                        stop=(s == HS - 1),
                    )
                l = work.tile([128, E], F32, tag="l")
                nc.vector.tensor_copy(l[:], pl[:])
                m1 = work.tile([128, 1], F32, tag="m1")
                nc.vector.reduce_max(m1[:], l[:], AX.X)
                mask1 = work.tile([128, E], F32, tag="mask1")
                nc.vector.tensor_tensor(
                    mask1[:], l[:], m1.to_broadcast([128, E]), ALU.is_equal
                )
                pen = work.tile([128, E], F32, tag="pen")
                nc.vector.tensor_scalar_mul(pen[:], mask1[:], 1.0e30)
                lm = work.tile([128, E], F32, tag="lm")
                nc.vector.tensor_sub(lm[:], l[:], pen[:])
                m2 = work.tile([128, 1], F32, tag="m2")
                nc.vector.reduce_max(m2[:], lm[:], AX.X)
                mask2 = work.tile([128, E], F32, tag="mask2")
                nc.vector.tensor_tensor(
                    mask2[:], lm[:], m2.to_broadcast([128, E]), ALU.is_equal
                )
                d = work.tile([128, 1], F32, tag="d")
                nc.vector.tensor_sub(d[:], m1[:], m2[:])
                w1 = work.tile([128, 1], F32, tag="w1")
                nc.scalar.activation(w1[:], d[:], AF.Sigmoid)
                w2 = work.tile([128, 1], F32, tag="w2")
                nc.vector.tensor_scalar(w2[:], w1[:], -1.0, 1.0, ALU.mult, ALU.add)
                t1 = work.tile([128, E], F32, tag="t1")
                nc.vector.tensor_tensor(
                    t1[:], mask1[:], w1.to_broadcast([128, E]), ALU.mult
                )
                t2 = work.tile([128, E], F32, tag="t2")
                nc.vector.tensor_tensor(
                    t2[:], mask2[:], w2.to_broadcast([128, E]), ALU.mult
                )
                wdt = work.tile([128, E], F32, tag="wdt")
                nc.vector.tensor_add(wdt[:], t1[:], t2[:])
                nc.sync.dma_start(
                    out=wd[tb * 128 : (tb + 1) * 128, :], in_=wdt[:]
                )
    return nc


def build_expert(cap: int = CAP) -> bass.Bass:
    """Per-core SwiGLU for one expert over CAP gathered tokens (bf16 matmuls,
    fp32 accumulate):  y^T = w * (silu(xe @ W1) * (xe @ W3)) @ W2, xe = x + dom.

    Inputs:  xt   [128, HS, CAP]    bf16  (xt[p, s, c]  = x_sel[c, s*128+p])
             w1t  [128, IS, HS, 128] bf16 (w1t[p, i, s, k] = W1[s*128+p, i*128+k])
             w3t  same layout as w1t
             w2t  [128, IS, HS, 128] bf16 (w2t[p, j, t, k] = W2[j*128+p, t*128+k])
             dom  [128, HS]          bf16 (dom[p, s] = dom_e[s*128+p])
             wrep [128, CAP]         f32  (combine weight, replicated over partitions)
    Output:  yt   [H, CAP] f32  (yt[h, c] = y_sel[c, h])
    """
    nc = bass.Bass()
    xt = nc.dram_tensor("xt", [128, HS, cap], BF16, kind="ExternalInput")
    w1t = nc.dram_tensor("w1t", [128, IS, HS, 128], BF16, kind="ExternalInput")
    w3t = nc.dram_tensor("w3t", [128, IS, HS, 128], BF16, kind="ExternalInput")
    w2t = nc.dram_tensor("w2t", [128, IS, HS, 128], BF16, kind="ExternalInput")
    dom = nc.dram_tensor("dom", [128, HS], BF16, kind="ExternalInput")
    wrep = nc.dram_tensor("wrep", [128, cap], F32, kind="ExternalInput")
    yt = nc.dram_tensor("yt", [H, cap], F32, kind="ExternalOutput")

    with _TileContext(nc) as tc:
        with (
            tc.tile_pool(name="const", bufs=1) as const,
            tc.tile_pool(name="wstream", bufs=4) as wstream,
            tc.tile_pool(name="hpool", bufs=1) as hpool,
            tc.tile_pool(name="work", bufs=4) as work,
            tc.tile_pool(name="ps_ab", bufs=2, space="PSUM") as ps_ab,
            tc.tile_pool(name="ps_y", bufs=2, space="PSUM") as ps_y,
        ):
            # startup-critical DMAs first: xe feeds the very first matmuls
            dom_sb = const.tile([128, HS], BF16, tag="dom")
            nc.sync.dma_start(out=dom_sb[:], in_=dom[:])
            xe_s = []
            for s in range(HS):
                xe = const.tile([128, cap], BF16, tag=f"xe{s}")
                nc.sync.dma_start(out=xe[:], in_=xt[:, s, :])
                nc.vector.tensor_tensor(
                    xe[:],
                    xe[:],
                    dom_sb[:, s : s + 1].to_broadcast([128, cap]),
                    ALU.add,
                )
                xe_s.append(xe)
            wr_sb = const.tile([128, cap], F32, tag="wrep")
            nc.sync.dma_start(out=wr_sb[:], in_=wrep[:])
            # W2 is first needed by phase 2 (~130us in); stream it in chunks
            # interleaved with the first tile's phase-1 slabs to keep it off
            # the startup critical path.
            w2_sb = const.tile([128, IS, HS, 128], BF16, tag="w2")

            for tile_idx, (t0, tt) in enumerate(_t_tiles(cap)):
                h_sb = hpool.tile([128, IS, 512], BF16, tag="h")
                # phase 1: a = xe @ W1, b = xe @ W3, h = silu(a) * b
                for it in range(IS):
                    w1_sb = wstream.tile([128, HS, 128], BF16, tag="w1")
                    nc.sync.dma_start(out=w1_sb[:], in_=w1t[:, it, :, :])
                    w3_sb = wstream.tile([128, HS, 128], BF16, tag="w3")
                    nc.sync.dma_start(out=w3_sb[:], in_=w3t[:, it, :, :])
                    if tile_idx == 0:
                        nc.sync.dma_start(
                            out=w2_sb[:, it, :, :], in_=w2t[:, it, :, :]
                        )
                    pa = ps_ab.tile([128, 512], F32, tag="pa")
                    pb = ps_ab.tile([128, 512], F32, tag="pb")
                    for s in range(HS):
                        nc.tensor.matmul(
                            pa[:, :tt],
                            lhsT=w1_sb[:, s, :],
                            rhs=xe_s[s][:, t0 : t0 + tt],
                            start=(s == 0),
                            stop=(s == HS - 1),
                        )
                    for s in range(HS):
                        nc.tensor.matmul(
                            pb[:, :tt],
                            lhsT=w3_sb[:, s, :],
                            rhs=xe_s[s][:, t0 : t0 + tt],
                            start=(s == 0),
                            stop=(s == HS - 1),
                        )
                    sa = work.tile([128, 512], F32, tag="sa")
                    nc.scalar.activation(sa[:, :tt], pa[:, :tt], AF.Silu)
                    nc.vector.tensor_tensor(
                        h_sb[:, it, :tt], sa[:, :tt], pb[:, :tt], ALU.mult
                    )
                # phase 2: y^T tile = w * (h @ W2)^T
                for ht in range(HS):
                    py = ps_y.tile([128, 512], F32, tag="py")
                    for j in range(IS):
                        nc.tensor.matmul(
                            py[:, :tt],
                            lhsT=w2_sb[:, j, ht, :],
                            rhs=h_sb[:, j, :tt],
                            start=(j == 0),
                            stop=(j == IS - 1),
                        )
                    yo = work.tile([128, 512], F32, tag="yo")
                    nc.vector.tensor_tensor(
                        yo[:, :tt], py[:, :tt], wr_sb[:, t0 : t0 + tt], ALU.mult
                    )
                    nc.sync.dma_start(
                        out=yt[ht * 128 : (ht + 1) * 128, t0 : t0 + tt],
                        in_=yo[:, :tt],
                    )
    return nc


_PROGRAMS: dict = {}


def _get_program(name, cap=CAP):
    key = (name, cap)
    if key not in _PROGRAMS:
        _PROGRAMS[key] = build_router() if name == "router" else build_expert(cap)
    return _PROGRAMS[key]


def _hs_split(a):
    """[D0, ...] with D0 = s*128+p  ->  [128, HS, ...] with [p, s, ...]."""
    return np.ascontiguousarray(
        a.reshape(HS, 128, *a.shape[1:]).swapaxes(0, 1)
    )


def kernel(hidden_states, gate_w, W1, W2, W3, dom):
    B, S, Hd = hidden_states.shape
    x2d = np.ascontiguousarray(
        np.asarray(hidden_states, dtype=np.float32).reshape(-1, Hd)
    )
    gate_w = np.asarray(gate_w, dtype=np.float32)
    W1 = np.asarray(W1, dtype=np.float32)
    W2 = np.asarray(W2, dtype=np.float32)
    W3 = np.asarray(W3, dtype=np.float32)
    dom = np.asarray(dom, dtype=np.float32)

    # ---- launch 1: router -------------------------------------------------
    gw_host = _hs_split(gate_w)  # [128, HS, E]
    in_maps1 = []
    for c in range(8):
        xs = x2d[c * TPC : (c + 1) * TPC]              # [TPC, H]
        xt = _hs_split(np.ascontiguousarray(xs.T))      # [128, HS, TPC]
        in_maps1.append({"xt": xt, "gw": gw_host})
    res1 = run_bass_kernel_spmd(_get_program("router"), in_maps1, list(range(8)))
    wd = np.concatenate([res1.results[c]["wd"] for c in range(8)], axis=0)  # [T, E]

    # ---- host dispatch ----------------------------------------------------
    x_bf = x2d.astype(NP_BF16)
    idxs = [np.nonzero(wd[:, e])[0] for e in range(E)]
    nsel = [len(idx) for idx in idxs]
    # fixed capacity normally; rebuild wider (multiple of 128) if ever exceeded
    cap = max(CAP, -(-max(nsel) // 128) * 128)
    in_maps2 = []
    for e in range(E):
        idx = idxs[e]
        n = nsel[e]
        pad_idx = np.zeros(cap, dtype=np.int64)
        pad_idx[:n] = idx
        w_sel = np.zeros(cap, dtype=np.float32)
        w_sel[:n] = wd[idx, e]

        xsel = x_bf[pad_idx]                            # [CAP, H]
        xt = _hs_split(np.ascontiguousarray(xsel.T))    # [128, HS, CAP] bf16
        w1t = np.ascontiguousarray(
            W1[e].astype(NP_BF16).reshape(HS, 128, IS, 128).transpose(1, 2, 0, 3)
        )
        w3t = np.ascontiguousarray(
            W3[e].astype(NP_BF16).reshape(HS, 128, IS, 128).transpose(1, 2, 0, 3)
        )
        w2t = np.ascontiguousarray(
            W2[e].astype(NP_BF16).reshape(IS, 128, HS, 128).transpose(1, 0, 2, 3)
        )
        dom_t = np.ascontiguousarray(dom[e].astype(NP_BF16).reshape(HS, 128).T)
        wrep = np.ascontiguousarray(np.broadcast_to(w_sel, (128, cap)))
        in_maps2.append(
            {"xt": xt, "w1t": w1t, "w3t": w3t, "w2t": w2t, "dom": dom_t,
             "wrep": wrep}
        )

    # ---- launch 2: experts ------------------------------------------------
    res2 = run_bass_kernel_spmd(_get_program("expert", cap), in_maps2, list(range(8)))

    # ---- host combine -----------------------------------------------------
    out = np.zeros((T, Hd), dtype=np.float32)
    for e in range(E):
        n = nsel[e]
        if n:
            yt = res2.results[e]["yt"]                  # [H, CAP] f32
            out[idxs[e]] += yt[:, :n].T
    return out.reshape(B, S, Hd)


# revision 30
# speedup vs baseline: 1.0263x; 1.0263x over previous
"""MoE layer (8 experts, top-2 routing, SwiGLU) on 8 Trainium2 NeuronCores.

Strategy (expert-parallel, capacity-based sparse dispatch):
  Launch 1 (router, data-parallel over tokens): each core computes fp32
    router logits for its 1024-token shard and emits the dense [T,8]
    combine-weight matrix (top-2 softmax weights, exact zeros elsewhere).
  Host: builds per-expert token index lists from the exact zero pattern,
    pads to a fixed capacity, gathers token columns (bf16) per expert.
  Launch 2 (experts, one expert per core): each core runs the SwiGLU MLP
    for its expert over its gathered tokens in bf16 (fp32 accumulate),
    scales by the combine weight, and returns y^T [H, CAP].
  Host: scatter-adds the per-expert outputs into the full [B,S,H] result.
"""

import numpy as np
import ml_dtypes

import concourse.bass as bass
import concourse.mybir as mybir
import concourse.tile as tile
from concourse.bass_utils import run_bass_kernel_spmd
from concourse.vector_clock import ScopedClock

BF16 = mybir.dt.bfloat16
F32 = mybir.dt.float32
AF = mybir.ActivationFunctionType
ALU = mybir.AluOpType
AX = mybir.AxisListType

H = 1024
I = 4096
E = 8
T = 8192
TPC = T // 8          # tokens per core in the router launch
CAP = 2192            # per-expert token capacity (max observed load 2182);
                      # overflow falls back to a wider rebuilt program
HS = H // 128         # 8 H sub-tiles
IS = I // 128         # 32 I sub-tiles
NP_BF16 = ml_dtypes.bfloat16


def _t_tiles(cap):
    """Split cap into equal-width (<=512) token tiles; PSUM bank = 512 fp32.
    Equal widths keep every tile's phase-1 PE work well above its fixed
    16MB W1/W3 slab traffic (a narrow remainder tile goes DMA-bound)."""
    n = -(-cap // 512)
    base, extra = divmod(cap, n)
    tiles, t0 = [], 0
    for i in range(n):
        tt = base + (1 if i < extra else 0)
        tiles.append((t0, tt))
        t0 += tt
    return tiles


_MAX_WAITS = 1  # this walrus build rejects multiple sync waits on one instruction


class _TileContext(tile.TileContext):
    """TileContext that hoists excess per-instruction semaphore waits into
    standalone same-engine nops; the walrus build here caps the number of
    sync waits a single instruction may carry."""

    def _add_instruction(self, inst):
        si = getattr(inst, "sync_info", None)
        if (
            si is not None
            and len(si.on_wait) > _MAX_WAITS
            and inst.engine != mybir.EngineType.Unassigned
        ):
            waits = list(si.on_wait)
            hoist, keep = waits[:-_MAX_WAITS], waits[-_MAX_WAITS:]
            for k in range(0, len(hoist), _MAX_WAITS):
                nop = mybir.InstNoOp(
                    name=self.nc.get_next_instruction_name(), ins=[], outs=[]
                )
                nop.engine = inst.engine
                nop.sync_info = mybir.SyncInfo(
                    on_wait=hoist[k : k + _MAX_WAITS], on_update=[]
                )
                super()._add_instruction(nop)
            si.on_wait = keep
        super()._add_instruction(inst)

    def _drain_and_barrier(self, tick_clock, wait_clock):
        nc = self.nc
        probe = nc.sync.nop(nofuse=True)
        wait_clock.add_sem_waits(
            probe.ins, ScopedClock({None: tick_clock.global_clock})
        )
        si = probe.ins.sync_info
        waits = list(si.on_wait) if si is not None else []
        if si is not None:
            si.on_wait = waits[:_MAX_WAITS]
        for k in range(_MAX_WAITS, len(waits), _MAX_WAITS):
            n = nc.sync.nop(nofuse=True)
            n.ins.sync_info = mybir.SyncInfo(
                on_wait=waits[k : k + _MAX_WAITS], on_update=[]
            )
        nc.sync.drain()
        nc.all_engine_barrier()
        popped = nc._tile_sem_poison_stack.pop()
        assert popped is self._sem_poison
        nc.clear_and_free_semaphores(list(self.sems.allocated().values()))
        nc.all_engine_barrier()


def build_router() -> bass.Bass:
    """Per-core: logits = x @ gate_w in fp32, top-2 softmax -> dense [TPC, E]
    combine weights (exact 0 for unselected experts).

    Inputs:  xt [128, HS, TPC] fp32  (xt[p, s, t] = x[t, s*128+p])
             gw [128, HS, E]  fp32  (gw[p, s, e] = gate_w[s*128+p, e])
    Output:  wd [TPC, E] fp32
    """
    nc = bass.Bass()
    xt = nc.dram_tensor("xt", [128, HS, TPC], F32, kind="ExternalInput")
    gw = nc.dram_tensor("gw", [128, HS, E], F32, kind="ExternalInput")
    wd = nc.dram_tensor("wd", [TPC, E], F32, kind="ExternalOutput")

    with _TileContext(nc) as tc:
        with (
            tc.tile_pool(name="const", bufs=1) as const,
            tc.tile_pool(name="work", bufs=8) as work,
            tc.tile_pool(name="psum", bufs=6, space="PSUM") as psum,
        ):
            gw_sb = const.tile([128, HS, E], F32, tag="gw")
            nc.sync.dma_start(out=gw_sb[:], in_=gw[:])

            for tb in range(TPC // 128):
                xt_sb = work.tile([128, HS, 128], F32, tag="xt")
                nc.sync.dma_start(
                    out=xt_sb[:], in_=xt[:, :, tb * 128 : (tb + 1) * 128]
                )
                pl = psum.tile([128, E], F32, tag="pl")
                for s in range(HS):
                    nc.tensor.matmul(
                        pl[:],
                        lhsT=xt_sb[:, s, :],
                        rhs=gw_sb[:, s, :],
                        start=(s == 0),
                        stop=(s == HS - 1),
                    )
                l = work.tile([128, E], F32, tag="l")
                nc.vector.tensor_copy(l[:], pl[:])
                m1 = work.tile([128, 1], F32, tag="m1")
                nc.vector.reduce_max(m1[:], l[:], AX.X)
                mask1 = work.tile([128, E], F32, tag="mask1")
                nc.vector.tensor_tensor(
                    mask1[:], l[:], m1.to_broadcast([128, E]), ALU.is_equal
                )
                pen = work.tile([128, E], F32, tag="pen")
                nc.vector.tensor_scalar_mul(pen[:], mask1[:], 1.0e30)
                lm = work.tile([128, E], F32, tag="lm")
                nc.vector.tensor_sub(lm[:], l[:], pen[:])
                m2 = work.tile([128, 1], F32, tag="m2")
                nc.vector.reduce_max(m2[:], lm[:], AX.X)
                mask2 = work.tile([128, E], F32, tag="mask2")
                nc.vector.tensor_tensor(
                    mask2[:], lm[:], m2.to_broadcast([128, E]), ALU.is_equal
                )
                d = work.tile([128, 1], F32, tag="d")
                nc.vector.tensor_sub(d[:], m1[:], m2[:])
                w1 = work.tile([128, 1], F32, tag="w1")
                nc.scalar.activation(w1[:], d[:], AF.Sigmoid)
                w2 = work.tile([128, 1], F32, tag="w2")
                nc.vector.tensor_scalar(w2[:], w1[:], -1.0, 1.0, ALU.mult, ALU.add)
                t1 = work.tile([128, E], F32, tag="t1")
                nc.vector.tensor_tensor(
                    t1[:], mask1[:], w1.to_broadcast([128, E]), ALU.mult
                )
                t2 = work.tile([128, E], F32, tag="t2")
                nc.vector.tensor_tensor(
                    t2[:], mask2[:], w2.to_broadcast([128, E]), ALU.mult
                )
                wdt = work.tile([128, E], F32, tag="wdt")
                nc.vector.tensor_add(wdt[:], t1[:], t2[:])
                nc.sync.dma_start(
                    out=wd[tb * 128 : (tb + 1) * 128, :], in_=wdt[:]
                )
    return nc


def build_expert(cap: int = CAP) -> bass.Bass:
    """Per-core SwiGLU for one expert over CAP gathered tokens (bf16 matmuls,
    fp32 accumulate):  y^T = w * (silu(xe @ W1) * (xe @ W3)) @ W2, xe = x + dom.

    Inputs:  xt   [128, HS, CAP]    bf16  (xt[p, s, c]  = x_sel[c, s*128+p])
             w13t [128, IS, 2, HS, 128] bf16 (w13t[p,i,0,s,k] = W1[s*128+p, i*128+k];
                                        w13t[p,i,1,s,k] = W3[...])
             w2t  [128, IS, HS, 128] bf16 (w2t[p, j, t, k] = W2[j*128+p, t*128+k])
             dom  [128, HS]          bf16 (dom[p, s] = dom_e[s*128+p])
             wrep [128, CAP]         f32  (combine weight, replicated over partitions)
    Output:  yt   [H, CAP] f32  (yt[h, c] = y_sel[c, h])
    """
    nc = bass.Bass()
    xt = nc.dram_tensor("xt", [128, HS, cap], BF16, kind="ExternalInput")
    w13t = nc.dram_tensor("w13t", [128, IS, 2, HS, 128], BF16, kind="ExternalInput")
    w2t = nc.dram_tensor("w2t", [128, IS, HS, 128], BF16, kind="ExternalInput")
    dom = nc.dram_tensor("dom", [128, HS], BF16, kind="ExternalInput")
    wrep = nc.dram_tensor("wrep", [128, cap], F32, kind="ExternalInput")
    yt = nc.dram_tensor("yt", [H, cap], F32, kind="ExternalOutput")

    with _TileContext(nc) as tc:
        with (
            tc.tile_pool(name="const", bufs=1) as const,
            tc.tile_pool(name="wstream", bufs=6) as wstream,
            tc.tile_pool(name="hpool", bufs=1) as hpool,
            tc.tile_pool(name="work", bufs=4) as work,
            tc.tile_pool(name="ps_ab", bufs=3, space="PSUM") as ps_ab,
            tc.tile_pool(name="ps_y", bufs=2, space="PSUM") as ps_y,
        ):
            # startup-critical DMAs first: xe feeds the very first matmuls
            dom_sb = const.tile([128, HS], BF16, tag="dom")
            nc.sync.dma_start(out=dom_sb[:], in_=dom[:])

            # PE warm-up: ~5us of garbage matmuls during the input DMA so the
            # HAM clock gate reaches 2.4 GHz before the real stream begins.
            wu = const.tile([128, 512], BF16, tag="warmup")
            nc.vector.memset(wu[:], 0)
            wu_ps = ps_y.tile([128, 512], F32, tag="py")
            for i in range(20):
                nc.tensor.matmul(
                    wu_ps[:],
                    lhsT=wu[:, :128],
                    rhs=wu[:],
                    start=(i == 0),
                    stop=(i == 19),
                )
            # DMA transfers drain in dispatch order, so interleave the first
            # I-tiles' W1/W3 slabs with the first xe sub-tiles: the PE gets
            # work as soon as each (slab, xe chunk) pair lands.
            NI = 3  # I-tile groups interleaved s-major during the xe fill
            pre_slabs = []
            xe_s = []
            for s in range(HS):
                if s < NI:
                    w13_sb = wstream.tile([128, 2, HS, 128], BF16, tag="w13")
                    nc.sync.dma_start(out=w13_sb[:], in_=w13t[:, s, :, :, :])
                    pre_slabs.append(w13_sb)
                xe = const.tile([128, cap], BF16, tag=f"xe{s}")
                nc.sync.dma_start(out=xe[:], in_=xt[:, s, :])
                nc.vector.tensor_tensor(
                    xe[:],
                    xe[:],
                    dom_sb[:, s : s + 1].to_broadcast([128, cap]),
                    ALU.add,
                )
                xe_s.append(xe)
            # wrep and W2 are first needed by phase 2 (~140us in); emitted
            # later (inside the first tile's loop) to keep them off the
            # startup-critical DMA window.
            wr_sb = const.tile([128, cap], F32, tag="wrep")
            w2_sb = const.tile([128, IS, HS, 128], BF16, tag="w2")

            def phase1_group(pa, pb, it, t0, tt, h_sb):
                sa = work.tile([128, 512], F32, tag="sa")
                nc.scalar.activation(sa[:, :tt], pa[:, :tt], AF.Silu)
                nc.vector.tensor_tensor(
                    h_sb[:, it, :tt], sa[:, :tt], pb[:, :tt], ALU.mult
                )

            for tile_idx, (t0, tt) in enumerate(_t_tiles(cap)):
                h_sb = hpool.tile([128, IS, 512], BF16, tag="h")
                # phase 1: a = xe @ W1, b = xe @ W3, h = silu(a) * b
                if tile_idx == 0:
                    # s-major across NI open PSUM groups: consume each xe
                    # sub-tile as its DMA lands instead of stalling on the
                    # full transfer.
                    pas, pbs = [], []
                    for k in range(NI):
                        pa = ps_ab.tile([128, 512], F32, tag="pa", name=f"pa0_{k}")
                        pb = ps_ab.tile([128, 512], F32, tag="pb", name=f"pb0_{k}")
                        pas.append(pa)
                        pbs.append(pb)
                    for s in range(HS):
                        for k in range(NI):
                            nc.tensor.matmul(
                                pas[k][:, :tt],
                                lhsT=pre_slabs[k][:, 0, s, :],
                                rhs=xe_s[s][:, t0 : t0 + tt],
                                start=(s == 0),
                                stop=(s == HS - 1),
                            )
                            nc.tensor.matmul(
                                pbs[k][:, :tt],
                                lhsT=pre_slabs[k][:, 1, s, :],
                                rhs=xe_s[s][:, t0 : t0 + tt],
                                start=(s == 0),
                                stop=(s == HS - 1),
                            )
                    for k in range(NI):
                        phase1_group(pas[k], pbs[k], k, t0, tt, h_sb)
                for it in range(NI if tile_idx == 0 else 0, IS):
                    w13_sb = wstream.tile([128, 2, HS, 128], BF16, tag="w13")
                    nc.sync.dma_start(out=w13_sb[:], in_=w13t[:, it, :, :, :])
                    if tile_idx == 0:
                        # wrep/W2 first used by phase 2; emit past the
                        # slot-recycle point so their transfers stay out of
                        # the xe fill window.
                        if it == 2 * NI:
                            nc.sync.dma_start(out=wr_sb[:], in_=wrep[:])
                        if it >= 2 * NI:
                            nc.sync.dma_start(
                                out=w2_sb[:, it, :, :], in_=w2t[:, it, :, :]
                            )
                        if it == IS - 1:
                            for j in range(2 * NI):
                                nc.sync.dma_start(
                                    out=w2_sb[:, j, :, :], in_=w2t[:, j, :, :]
                                )
                    pa = ps_ab.tile([128, 512], F32, tag="pa")
                    pb = ps_ab.tile([128, 512], F32, tag="pb")
                    for s in range(HS):
                        nc.tensor.matmul(
                            pa[:, :tt],
                            lhsT=w13_sb[:, 0, s, :],
                            rhs=xe_s[s][:, t0 : t0 + tt],
                            start=(s == 0),
                            stop=(s == HS - 1),
                        )
                    for s in range(HS):
                        nc.tensor.matmul(
                            pb[:, :tt],
                            lhsT=w13_sb[:, 1, s, :],
                            rhs=xe_s[s][:, t0 : t0 + tt],
                            start=(s == 0),
                            stop=(s == HS - 1),
                        )
                    phase1_group(pa, pb, it, t0, tt, h_sb)
                # phase 2: y^T tile = w * (h @ W2)^T
                for ht in range(HS):
                    py = ps_y.tile([128, 512], F32, tag="py")
                    for j in range(IS):
                        nc.tensor.matmul(
                            py[:, :tt],
                            lhsT=w2_sb[:, j, ht, :],
                            rhs=h_sb[:, j, :tt],
                            start=(j == 0),
                            stop=(j == IS - 1),
                        )
                    yo = work.tile([128, 512], F32, tag="yo")
                    nc.vector.tensor_tensor(
                        yo[:, :tt], py[:, :tt], wr_sb[:, t0 : t0 + tt], ALU.mult
                    )
                    nc.sync.dma_start(
                        out=yt[ht * 128 : (ht + 1) * 128, t0 : t0 + tt],
                        in_=yo[:, :tt],
                    )
    return nc


_PROGRAMS: dict = {}


def _get_program(name, cap=CAP):
    key = (name, cap)
    if key not in _PROGRAMS:
        _PROGRAMS[key] = build_router() if name == "router" else build_expert(cap)
    return _PROGRAMS[key]


def _hs_split(a):
    """[D0, ...] with D0 = s*128+p  ->  [128, HS, ...] with [p, s, ...]."""
    return np.ascontiguousarray(
        a.reshape(HS, 128, *a.shape[1:]).swapaxes(0, 1)
    )


def kernel(hidden_states, gate_w, W1, W2, W3, dom):
    B, S, Hd = hidden_states.shape
    x2d = np.ascontiguousarray(
        np.asarray(hidden_states, dtype=np.float32).reshape(-1, Hd)
    )
    gate_w = np.asarray(gate_w, dtype=np.float32)
    W1 = np.asarray(W1, dtype=np.float32)
    W2 = np.asarray(W2, dtype=np.float32)
    W3 = np.asarray(W3, dtype=np.float32)
    dom = np.asarray(dom, dtype=np.float32)

    # ---- launch 1: router -------------------------------------------------
    gw_host = _hs_split(gate_w)  # [128, HS, E]
    in_maps1 = []
    for c in range(8):
        xs = x2d[c * TPC : (c + 1) * TPC]              # [TPC, H]
        xt = _hs_split(np.ascontiguousarray(xs.T))      # [128, HS, TPC]
        in_maps1.append({"xt": xt, "gw": gw_host})
    res1 = run_bass_kernel_spmd(_get_program("router"), in_maps1, list(range(8)))
    wd = np.concatenate([res1.results[c]["wd"] for c in range(8)], axis=0)  # [T, E]

    # ---- host dispatch ----------------------------------------------------
    x_bf = x2d.astype(NP_BF16)
    idxs = [np.nonzero(wd[:, e])[0] for e in range(E)]
    nsel = [len(idx) for idx in idxs]
    # fixed capacity normally; rebuild wider (multiple of 128) if ever exceeded
    cap = max(CAP, -(-max(nsel) // 128) * 128)
    in_maps2 = []
    for e in range(E):
        idx = idxs[e]
        n = nsel[e]
        pad_idx = np.zeros(cap, dtype=np.int64)
        pad_idx[:n] = idx
        w_sel = np.zeros(cap, dtype=np.float32)
        w_sel[:n] = wd[idx, e]

        xsel = x_bf[pad_idx]                            # [CAP, H]
        xt = _hs_split(np.ascontiguousarray(xsel.T))    # [128, HS, CAP] bf16
        w1r = W1[e].astype(NP_BF16).reshape(HS, 128, IS, 128).transpose(1, 2, 0, 3)
        w3r = W3[e].astype(NP_BF16).reshape(HS, 128, IS, 128).transpose(1, 2, 0, 3)
        w13t = np.ascontiguousarray(np.stack([w1r, w3r], axis=2))
        w2t = np.ascontiguousarray(
            W2[e].astype(NP_BF16).reshape(IS, 128, HS, 128).transpose(1, 0, 2, 3)
        )
        dom_t = np.ascontiguousarray(dom[e].astype(NP_BF16).reshape(HS, 128).T)
        wrep = np.ascontiguousarray(np.broadcast_to(w_sel, (128, cap)))
        in_maps2.append(
            {"xt": xt, "w13t": w13t, "w2t": w2t, "dom": dom_t, "wrep": wrep}
        )

    # ---- launch 2: experts ------------------------------------------------
    res2 = run_bass_kernel_spmd(_get_program("expert", cap), in_maps2, list(range(8)))

    # ---- host combine -----------------------------------------------------
    out = np.zeros((T, Hd), dtype=np.float32)
    for e in range(E):
        n = nsel[e]
        if n:
            yt = res2.results[e]["yt"]                  # [H, CAP] f32
            out[idxs[e]] += yt[:, :n].T
    return out.reshape(B, S, Hd)


# revision 33
# speedup vs baseline: 1.0283x; 1.0020x over previous
"""MoE layer (8 experts, top-2 routing, SwiGLU) on 8 Trainium2 NeuronCores.

Strategy (expert-parallel, capacity-based sparse dispatch):
  Launch 1 (router, data-parallel over tokens): each core computes fp32
    router logits for its 1024-token shard and emits the dense [T,8]
    combine-weight matrix (top-2 softmax weights, exact zeros elsewhere).
  Host: builds per-expert token index lists from the exact zero pattern,
    pads to a fixed capacity, gathers token columns (bf16) per expert.
  Launch 2 (experts, one expert per core): each core runs the SwiGLU MLP
    for its expert over its gathered tokens in bf16 (fp32 accumulate),
    scales by the combine weight, and returns y^T [H, CAP].
  Host: scatter-adds the per-expert outputs into the full [B,S,H] result.
"""

import numpy as np
import ml_dtypes

import concourse.bass as bass
import concourse.mybir as mybir
import concourse.tile as tile
from concourse.bass_utils import run_bass_kernel_spmd
from concourse.vector_clock import ScopedClock

BF16 = mybir.dt.bfloat16
F32 = mybir.dt.float32
AF = mybir.ActivationFunctionType
ALU = mybir.AluOpType
AX = mybir.AxisListType

H = 1024
I = 4096
E = 8
T = 8192
TPC = T // 8          # tokens per core in the router launch
CAP = 2192            # per-expert token capacity (max observed load 2182);
                      # overflow falls back to a wider rebuilt program
HS = H // 128         # 8 H sub-tiles
IS = I // 128         # 32 I sub-tiles
NP_BF16 = ml_dtypes.bfloat16


def _t_tiles(cap):
    """Split cap into equal-width (<=512) token tiles; PSUM bank = 512 fp32.
    Equal widths keep every tile's phase-1 PE work well above its fixed
    16MB W1/W3 slab traffic (a narrow remainder tile goes DMA-bound)."""
    n = -(-cap // 512)
    base, extra = divmod(cap, n)
    tiles, t0 = [], 0
    for i in range(n):
        tt = base + (1 if i < extra else 0)
        tiles.append((t0, tt))
        t0 += tt
    return tiles


_MAX_WAITS = 1  # this walrus build rejects multiple sync waits on one instruction


class _TileContext(tile.TileContext):
    """TileContext that hoists excess per-instruction semaphore waits into
    standalone same-engine nops; the walrus build here caps the number of
    sync waits a single instruction may carry."""

    def _add_instruction(self, inst):
        si = getattr(inst, "sync_info", None)
        if (
            si is not None
            and len(si.on_wait) > _MAX_WAITS
            and inst.engine != mybir.EngineType.Unassigned
        ):
            waits = list(si.on_wait)
            hoist, keep = waits[:-_MAX_WAITS], waits[-_MAX_WAITS:]
            for k in range(0, len(hoist), _MAX_WAITS):
                nop = mybir.InstNoOp(
                    name=self.nc.get_next_instruction_name(), ins=[], outs=[]
                )
                nop.engine = inst.engine
                nop.sync_info = mybir.SyncInfo(
                    on_wait=hoist[k : k + _MAX_WAITS], on_update=[]
                )
                super()._add_instruction(nop)
            si.on_wait = keep
        super()._add_instruction(inst)

    def _drain_and_barrier(self, tick_clock, wait_clock):
        nc = self.nc
        probe = nc.sync.nop(nofuse=True)
        wait_clock.add_sem_waits(
            probe.ins, ScopedClock({None: tick_clock.global_clock})
        )
        si = probe.ins.sync_info
        waits = list(si.on_wait) if si is not None else []
        if si is not None:
            si.on_wait = waits[:_MAX_WAITS]
        for k in range(_MAX_WAITS, len(waits), _MAX_WAITS):
            n = nc.sync.nop(nofuse=True)
            n.ins.sync_info = mybir.SyncInfo(
                on_wait=waits[k : k + _MAX_WAITS], on_update=[]
            )
        nc.sync.drain()
        nc.all_engine_barrier()
        popped = nc._tile_sem_poison_stack.pop()
        assert popped is self._sem_poison
        nc.clear_and_free_semaphores(list(self.sems.allocated().values()))
        nc.all_engine_barrier()


def build_router() -> bass.Bass:
    """Per-core: logits = x @ gate_w in fp32, top-2 softmax -> dense [TPC, E]
    combine weights (exact 0 for unselected experts).

    Inputs:  xt [128, HS, TPC] fp32  (xt[p, s, t] = x[t, s*128+p])
             gw [128, HS, E]  fp32  (gw[p, s, e] = gate_w[s*128+p, e])
    Output:  wd [TPC, E] fp32
    """
    nc = bass.Bass()
    xt = nc.dram_tensor("xt", [128, HS, TPC], F32, kind="ExternalInput")
    gw = nc.dram_tensor("gw", [128, HS, E], F32, kind="ExternalInput")
    wd = nc.dram_tensor("wd", [TPC, E], F32, kind="ExternalOutput")

    with _TileContext(nc) as tc:
        with (
            tc.tile_pool(name="const", bufs=1) as const,
            tc.tile_pool(name="work", bufs=8) as work,
            tc.tile_pool(name="psum", bufs=6, space="PSUM") as psum,
        ):
            gw_sb = const.tile([128, HS, E], F32, tag="gw")
            nc.sync.dma_start(out=gw_sb[:], in_=gw[:])

            # one bulk transfer: the router is latency-bound, not
            # compute-bound, so per-DMA issue overheads dominate 8 small DMAs
            xt_sb = const.tile([128, HS, TPC], F32, tag="xtall")
            q = TPC // 4
            for k in range(4):
                nc.sync.dma_start(
                    out=xt_sb[:, :, k * q : (k + 1) * q],
                    in_=xt[:, :, k * q : (k + 1) * q],
                )

            # all 8 token blocks accumulate into one PSUM bank so the top-2
            # math runs ONCE on [128, NB, E] instead of 8x on [128, E]
            NB = TPC // 128
            pl = psum.tile([128, NB, E], F32, tag="pl")
            for tb in range(NB):
                for s in range(HS):
                    nc.tensor.matmul(
                        pl[:, tb, :],
                        lhsT=xt_sb[:, s, tb * 128 : (tb + 1) * 128],
                        rhs=gw_sb[:, s, :],
                        start=(s == 0),
                        stop=(s == HS - 1),
                    )
            l = work.tile([128, NB, E], F32, tag="l")
            nc.vector.tensor_copy(l[:], pl[:])
            m1 = work.tile([128, NB], F32, tag="m1")
            nc.vector.reduce_max(m1[:], l[:], AX.X)
            mask1 = work.tile([128, NB, E], F32, tag="mask1")
            nc.vector.tensor_tensor(
                mask1[:], l[:], m1[:, :, None].to_broadcast([128, NB, E]),
                ALU.is_equal,
            )
            pen = work.tile([128, NB, E], F32, tag="pen")
            nc.vector.tensor_scalar_mul(pen[:], mask1[:], 1.0e30)
            lm = work.tile([128, NB, E], F32, tag="lm")
            nc.vector.tensor_sub(lm[:], l[:], pen[:])
            m2 = work.tile([128, NB], F32, tag="m2")
            nc.vector.reduce_max(m2[:], lm[:], AX.X)
            mask2 = work.tile([128, NB, E], F32, tag="mask2")
            nc.vector.tensor_tensor(
                mask2[:], lm[:], m2[:, :, None].to_broadcast([128, NB, E]),
                ALU.is_equal,
            )
            d = work.tile([128, NB], F32, tag="d")
            nc.vector.tensor_sub(d[:], m1[:], m2[:])
            w1 = work.tile([128, NB], F32, tag="w1")
            nc.scalar.activation(w1[:], d[:], AF.Sigmoid)
            w2 = work.tile([128, NB], F32, tag="w2")
            nc.vector.tensor_scalar(w2[:], w1[:], -1.0, 1.0, ALU.mult, ALU.add)
            t1 = work.tile([128, NB, E], F32, tag="t1")
            nc.vector.tensor_tensor(
                t1[:], mask1[:], w1[:, :, None].to_broadcast([128, NB, E]),
                ALU.mult,
            )
            t2 = work.tile([128, NB, E], F32, tag="t2")
            nc.vector.tensor_tensor(
                t2[:], mask2[:], w2[:, :, None].to_broadcast([128, NB, E]),
                ALU.mult,
            )
            wdt = work.tile([128, NB, E], F32, tag="wdt")
            nc.vector.tensor_add(wdt[:], t1[:], t2[:])
            nc.sync.dma_start(
                out=wd.rearrange("(b p) e -> p b e", p=128), in_=wdt[:]
            )
    return nc


def build_expert(cap: int = CAP) -> bass.Bass:
    """Per-core SwiGLU for one expert over CAP gathered tokens (bf16 matmuls,
    fp32 accumulate):  y^T = w * (silu(xe @ W1) * (xe @ W3)) @ W2, xe = x + dom.

    Inputs:  xt   [128, HS, CAP]    bf16  (xt[p, s, c]  = x_sel[c, s*128+p])
             w13t [128, IS, 2, HS, 128] bf16 (w13t[p,i,0,s,k] = W1[s*128+p, i*128+k];
                                        w13t[p,i,1,s,k] = W3[...])
             w2t  [128, IS, HS, 128] bf16 (w2t[p, j, t, k] = W2[j*128+p, t*128+k])
             dom  [128, HS]          bf16 (dom[p, s] = dom_e[s*128+p])
             wrep [128, CAP]         f32  (combine weight, replicated over partitions)
    Output:  yt   [H, CAP] f32  (yt[h, c] = y_sel[c, h])
    """
    nc = bass.Bass()
    xt = nc.dram_tensor("xt", [128, HS, cap], BF16, kind="ExternalInput")
    w13t = nc.dram_tensor("w13t", [128, IS, 2, HS, 128], BF16, kind="ExternalInput")
    w2t = nc.dram_tensor("w2t", [128, IS, HS, 128], BF16, kind="ExternalInput")
    dom = nc.dram_tensor("dom", [128, HS], BF16, kind="ExternalInput")
    wrep = nc.dram_tensor("wrep", [128, cap], F32, kind="ExternalInput")
    yt = nc.dram_tensor("yt", [H, cap], F32, kind="ExternalOutput")

    with _TileContext(nc) as tc:
        with (
            tc.tile_pool(name="const", bufs=1) as const,
            tc.tile_pool(name="wstream", bufs=6) as wstream,
            tc.tile_pool(name="hpool", bufs=1) as hpool,
            tc.tile_pool(name="work", bufs=4) as work,
            tc.tile_pool(name="ps_ab", bufs=3, space="PSUM") as ps_ab,
            tc.tile_pool(name="ps_y", bufs=2, space="PSUM") as ps_y,
        ):
            # startup-critical DMAs first: xe feeds the very first matmuls
            dom_sb = const.tile([128, HS], BF16, tag="dom")
            nc.sync.dma_start(out=dom_sb[:], in_=dom[:])

            # PE warm-up: ~5us of garbage matmuls during the input DMA so the
            # HAM clock gate reaches 2.4 GHz before the real stream begins.
            wu = const.tile([128, 512], BF16, tag="warmup")
            nc.vector.memset(wu[:], 0)
            wu_ps = ps_y.tile([128, 512], F32, tag="py")
            for i in range(20):
                nc.tensor.matmul(
                    wu_ps[:],
                    lhsT=wu[:, :128],
                    rhs=wu[:],
                    start=(i == 0),
                    stop=(i == 19),
                )
            # DMA transfers drain in dispatch order, so interleave the first
            # I-tiles' W1/W3 slabs with the first xe sub-tiles: the PE gets
            # work as soon as each (slab, xe chunk) pair lands.
            NI = 3  # I-tile groups interleaved s-major during the xe fill
            pre_slabs = []
            xe_s = []
            for s in range(HS):
                if s < NI:
                    w13_sb = wstream.tile([128, 2, HS, 128], BF16, tag="w13")
                    nc.sync.dma_start(out=w13_sb[:], in_=w13t[:, s, :, :, :])
                    pre_slabs.append(w13_sb)
                xe = const.tile([128, cap], BF16, tag=f"xe{s}")
                nc.sync.dma_start(out=xe[:], in_=xt[:, s, :])
                nc.vector.tensor_tensor(
                    xe[:],
                    xe[:],
                    dom_sb[:, s : s + 1].to_broadcast([128, cap]),
                    ALU.add,
                )
                xe_s.append(xe)
            # wrep and W2 are first needed by phase 2 (~140us in); emitted
            # later (inside the first tile's loop) to keep them off the
            # startup-critical DMA window.
            wr_sb = const.tile([128, cap], F32, tag="wrep")
            w2_sb = const.tile([128, IS, HS, 128], BF16, tag="w2")

            def phase1_group(pa, pb, it, t0, tt, h_sb):
                sa = work.tile([128, 512], F32, tag="sa")
                nc.scalar.activation(sa[:, :tt], pa[:, :tt], AF.Silu)
                nc.vector.tensor_tensor(
                    h_sb[:, it, :tt], sa[:, :tt], pb[:, :tt], ALU.mult
                )

            for tile_idx, (t0, tt) in enumerate(_t_tiles(cap)):
                h_sb = hpool.tile([128, IS, 512], BF16, tag="h")
                # phase 1: a = xe @ W1, b = xe @ W3, h = silu(a) * b
                if tile_idx == 0:
                    # s-major across NI open PSUM groups: consume each xe
                    # sub-tile as its DMA lands instead of stalling on the
                    # full transfer.
                    pas, pbs = [], []
                    for k in range(NI):
                        pa = ps_ab.tile([128, 512], F32, tag="pa", name=f"pa0_{k}")
                        pb = ps_ab.tile([128, 512], F32, tag="pb", name=f"pb0_{k}")
                        pas.append(pa)
                        pbs.append(pb)
                    for s in range(HS):
                        for k in range(NI):
                            nc.tensor.matmul(
                                pas[k][:, :tt],
                                lhsT=pre_slabs[k][:, 0, s, :],
                                rhs=xe_s[s][:, t0 : t0 + tt],
                                start=(s == 0),
                                stop=(s == HS - 1),
                            )
                            nc.tensor.matmul(
                                pbs[k][:, :tt],
                                lhsT=pre_slabs[k][:, 1, s, :],
                                rhs=xe_s[s][:, t0 : t0 + tt],
                                start=(s == 0),
                                stop=(s == HS - 1),
                            )
                    for k in range(NI):
                        phase1_group(pas[k], pbs[k], k, t0, tt, h_sb)
                for it in range(NI if tile_idx == 0 else 0, IS):
                    w13_sb = wstream.tile([128, 2, HS, 128], BF16, tag="w13")
                    nc.sync.dma_start(out=w13_sb[:], in_=w13t[:, it, :, :, :])
                    if tile_idx == 0:
                        # wrep/W2 first used by phase 2; emit past the
                        # slot-recycle point so their transfers stay out of
                        # the xe fill window.
                        if it == 2 * NI:
                            nc.sync.dma_start(out=wr_sb[:], in_=wrep[:])
                        if it >= 2 * NI:
                            nc.sync.dma_start(
                                out=w2_sb[:, it, :, :], in_=w2t[:, it, :, :]
                            )
                        if it == IS - 1:
                            for j in range(2 * NI):
                                nc.sync.dma_start(
                                    out=w2_sb[:, j, :, :], in_=w2t[:, j, :, :]
                                )
                    pa = ps_ab.tile([128, 512], F32, tag="pa")
                    pb = ps_ab.tile([128, 512], F32, tag="pb")
                    for s in range(HS):
                        nc.tensor.matmul(
                            pa[:, :tt],
                            lhsT=w13_sb[:, 0, s, :],
                            rhs=xe_s[s][:, t0 : t0 + tt],
                            start=(s == 0),
                            stop=(s == HS - 1),
                        )
                    for s in range(HS):
                        nc.tensor.matmul(
                            pb[:, :tt],
                            lhsT=w13_sb[:, 1, s, :],
                            rhs=xe_s[s][:, t0 : t0 + tt],
                            start=(s == 0),
                            stop=(s == HS - 1),
                        )
                    phase1_group(pa, pb, it, t0, tt, h_sb)
                # phase 2: y^T tile = w * (h @ W2)^T
                for ht in range(HS):
                    py = ps_y.tile([128, 512], F32, tag="py")
                    for j in range(IS):
                        nc.tensor.matmul(
                            py[:, :tt],
                            lhsT=w2_sb[:, j, ht, :],
                            rhs=h_sb[:, j, :tt],
                            start=(j == 0),
                            stop=(j == IS - 1),
                        )
                    yo = work.tile([128, 512], F32, tag="yo")
                    nc.vector.tensor_tensor(
                        yo[:, :tt], py[:, :tt], wr_sb[:, t0 : t0 + tt], ALU.mult
                    )
                    nc.sync.dma_start(
                        out=yt[ht * 128 : (ht + 1) * 128, t0 : t0 + tt],
                        in_=yo[:, :tt],
                    )
    return nc


_PROGRAMS: dict = {}


def _get_program(name, cap=CAP):
    key = (name, cap)
    if key not in _PROGRAMS:
        _PROGRAMS[key] = build_router() if name == "router" else build_expert(cap)
    return _PROGRAMS[key]


def _hs_split(a):
    """[D0, ...] with D0 = s*128+p  ->  [128, HS, ...] with [p, s, ...]."""
    return np.ascontiguousarray(
        a.reshape(HS, 128, *a.shape[1:]).swapaxes(0, 1)
    )


def kernel(hidden_states, gate_w, W1, W2, W3, dom):
    B, S, Hd = hidden_states.shape
    x2d = np.ascontiguousarray(
        np.asarray(hidden_states, dtype=np.float32).reshape(-1, Hd)
    )
    gate_w = np.asarray(gate_w, dtype=np.float32)
    W1 = np.asarray(W1, dtype=np.float32)
    W2 = np.asarray(W2, dtype=np.float32)
    W3 = np.asarray(W3, dtype=np.float32)
    dom = np.asarray(dom, dtype=np.float32)

    # ---- launch 1: router -------------------------------------------------
    gw_host = _hs_split(gate_w)  # [128, HS, E]
    in_maps1 = []
    for c in range(8):
        xs = x2d[c * TPC : (c + 1) * TPC]              # [TPC, H]
        xt = _hs_split(np.ascontiguousarray(xs.T))      # [128, HS, TPC]
        in_maps1.append({"xt": xt, "gw": gw_host})
    res1 = run_bass_kernel_spmd(_get_program("router"), in_maps1, list(range(8)))
    wd = np.concatenate([res1.results[c]["wd"] for c in range(8)], axis=0)  # [T, E]

    # ---- host dispatch ----------------------------------------------------
    x_bf = x2d.astype(NP_BF16)
    idxs = [np.nonzero(wd[:, e])[0] for e in range(E)]
    nsel = [len(idx) for idx in idxs]
    # fixed capacity normally; rebuild wider (multiple of 128) if ever exceeded
    cap = max(CAP, -(-max(nsel) // 128) * 128)
    in_maps2 = []
    for e in range(E):
        idx = idxs[e]
        n = nsel[e]
        pad_idx = np.zeros(cap, dtype=np.int64)
        pad_idx[:n] = idx
        w_sel = np.zeros(cap, dtype=np.float32)
        w_sel[:n] = wd[idx, e]

        xsel = x_bf[pad_idx]                            # [CAP, H]
        xt = _hs_split(np.ascontiguousarray(xsel.T))    # [128, HS, CAP] bf16
        w1r = W1[e].astype(NP_BF16).reshape(HS, 128, IS, 128).transpose(1, 2, 0, 3)
        w3r = W3[e].astype(NP_BF16).reshape(HS, 128, IS, 128).transpose(1, 2, 0, 3)
        w13t = np.ascontiguousarray(np.stack([w1r, w3r], axis=2))
        w2t = np.ascontiguousarray(
            W2[e].astype(NP_BF16).reshape(IS, 128, HS, 128).transpose(1, 0, 2, 3)
        )
        dom_t = np.ascontiguousarray(dom[e].astype(NP_BF16).reshape(HS, 128).T)
        wrep = np.ascontiguousarray(np.broadcast_to(w_sel, (128, cap)))
        in_maps2.append(
            {"xt": xt, "w13t": w13t, "w2t": w2t, "dom": dom_t, "wrep": wrep}
        )

    # ---- launch 2: experts ------------------------------------------------
    res2 = run_bass_kernel_spmd(_get_program("expert", cap), in_maps2, list(range(8)))

    # ---- host combine -----------------------------------------------------
    out = np.zeros((T, Hd), dtype=np.float32)
    for e in range(E):
        n = nsel[e]
        if n:
            yt = res2.results[e]["yt"]                  # [H, CAP] f32
            out[idxs[e]] += yt[:, :n].T
    return out.reshape(B, S, Hd)


# revision 34
# speedup vs baseline: 1.0309x; 1.0025x over previous
"""MoE layer (8 experts, top-2 routing, SwiGLU) on 8 Trainium2 NeuronCores.

Strategy (expert-parallel, capacity-based sparse dispatch):
  Launch 1 (router, data-parallel over tokens): each core computes fp32
    router logits for its 1024-token shard and emits the dense [T,8]
    combine-weight matrix (top-2 softmax weights, exact zeros elsewhere).
  Host: builds per-expert token index lists from the exact zero pattern,
    pads to a fixed capacity, gathers token columns (bf16) per expert.
  Launch 2 (experts, one expert per core): each core runs the SwiGLU MLP
    for its expert over its gathered tokens in bf16 (fp32 accumulate),
    scales by the combine weight, and returns y^T [H, CAP].
  Host: scatter-adds the per-expert outputs into the full [B,S,H] result.
"""

import numpy as np
import ml_dtypes

import concourse.bass as bass
import concourse.mybir as mybir
import concourse.tile as tile
from concourse.bass_utils import run_bass_kernel_spmd
from concourse.vector_clock import ScopedClock

BF16 = mybir.dt.bfloat16
F32 = mybir.dt.float32
AF = mybir.ActivationFunctionType
ALU = mybir.AluOpType
AX = mybir.AxisListType

H = 1024
I = 4096
E = 8
T = 8192
TPC = T // 8          # tokens per core in the router launch
CAP = 2192            # per-expert token capacity (max observed load 2182);
                      # overflow falls back to a wider rebuilt program
HS = H // 128         # 8 H sub-tiles
IS = I // 128         # 32 I sub-tiles
NP_BF16 = ml_dtypes.bfloat16


def _t_tiles(cap):
    """Split cap into equal-width (<=512) token tiles; PSUM bank = 512 fp32.
    Equal widths keep every tile's phase-1 PE work well above its fixed
    16MB W1/W3 slab traffic (a narrow remainder tile goes DMA-bound)."""
    n = -(-cap // 512)
    base, extra = divmod(cap, n)
    tiles, t0 = [], 0
    for i in range(n):
        tt = base + (1 if i < extra else 0)
        tiles.append((t0, tt))
        t0 += tt
    return tiles


_MAX_WAITS = 1  # this walrus build rejects multiple sync waits on one instruction


class _TileContext(tile.TileContext):
    """TileContext that hoists excess per-instruction semaphore waits into
    standalone same-engine nops; the walrus build here caps the number of
    sync waits a single instruction may carry."""

    def _add_instruction(self, inst):
        si = getattr(inst, "sync_info", None)
        if (
            si is not None
            and len(si.on_wait) > _MAX_WAITS
            and inst.engine != mybir.EngineType.Unassigned
        ):
            waits = list(si.on_wait)
            hoist, keep = waits[:-_MAX_WAITS], waits[-_MAX_WAITS:]
            for k in range(0, len(hoist), _MAX_WAITS):
                nop = mybir.InstNoOp(
                    name=self.nc.get_next_instruction_name(), ins=[], outs=[]
                )
                nop.engine = inst.engine
                nop.sync_info = mybir.SyncInfo(
                    on_wait=hoist[k : k + _MAX_WAITS], on_update=[]
                )
                super()._add_instruction(nop)
            si.on_wait = keep
        super()._add_instruction(inst)

    def _drain_and_barrier(self, tick_clock, wait_clock):
        nc = self.nc
        probe = nc.sync.nop(nofuse=True)
        wait_clock.add_sem_waits(
            probe.ins, ScopedClock({None: tick_clock.global_clock})
        )
        si = probe.ins.sync_info
        waits = list(si.on_wait) if si is not None else []
        if si is not None:
            si.on_wait = waits[:_MAX_WAITS]
        for k in range(_MAX_WAITS, len(waits), _MAX_WAITS):
            n = nc.sync.nop(nofuse=True)
            n.ins.sync_info = mybir.SyncInfo(
                on_wait=waits[k : k + _MAX_WAITS], on_update=[]
            )
        nc.sync.drain()
        nc.all_engine_barrier()
        popped = nc._tile_sem_poison_stack.pop()
        assert popped is self._sem_poison
        nc.clear_and_free_semaphores(list(self.sems.allocated().values()))
        nc.all_engine_barrier()


def build_router() -> bass.Bass:
    """Per-core: logits = x @ gate_w in fp32, top-2 softmax -> dense [TPC, E]
    combine weights (exact 0 for unselected experts).

    Inputs:  xt [128, HS, TPC] fp32  (xt[p, s, t] = x[t, s*128+p])
             gw [128, HS, E]  fp32  (gw[p, s, e] = gate_w[s*128+p, e])
    Output:  wd [TPC, E] fp32
    """
    nc = bass.Bass()
    xt = nc.dram_tensor("xt", [128, HS, TPC], F32, kind="ExternalInput")
    gw = nc.dram_tensor("gw", [128, HS, E], F32, kind="ExternalInput")
    wd = nc.dram_tensor("wd", [TPC, E], F32, kind="ExternalOutput")

    with _TileContext(nc) as tc:
        with (
            tc.tile_pool(name="const", bufs=1) as const,
            tc.tile_pool(name="work", bufs=8) as work,
            tc.tile_pool(name="psum", bufs=6, space="PSUM") as psum,
        ):
            gw_sb = const.tile([128, HS, E], F32, tag="gw")
            nc.sync.dma_start(out=gw_sb[:], in_=gw[:])

            # one bulk transfer: the router is latency-bound, not
            # compute-bound, so per-DMA issue overheads dominate 8 small DMAs
            xt_sb = const.tile([128, HS, TPC], F32, tag="xtall")
            q = TPC // 4
            for k in range(4):
                nc.sync.dma_start(
                    out=xt_sb[:, :, k * q : (k + 1) * q],
                    in_=xt[:, :, k * q : (k + 1) * q],
                )

            # all 8 token blocks accumulate into one PSUM bank so the top-2
            # math runs ONCE on [128, NB, E] instead of 8x on [128, E]
            NB = TPC // 128
            pl = psum.tile([128, NB, E], F32, tag="pl")
            for tb in range(NB):
                for s in range(HS):
                    nc.tensor.matmul(
                        pl[:, tb, :],
                        lhsT=xt_sb[:, s, tb * 128 : (tb + 1) * 128],
                        rhs=gw_sb[:, s, :],
                        start=(s == 0),
                        stop=(s == HS - 1),
                    )
            l = work.tile([128, NB, E], F32, tag="l")
            nc.vector.tensor_copy(l[:], pl[:])
            m1 = work.tile([128, NB], F32, tag="m1")
            nc.vector.reduce_max(m1[:], l[:], AX.X)
            mask1 = work.tile([128, NB, E], F32, tag="mask1")
            nc.vector.tensor_tensor(
                mask1[:], l[:], m1[:, :, None].to_broadcast([128, NB, E]),
                ALU.is_equal,
            )
            pen = work.tile([128, NB, E], F32, tag="pen")
            nc.vector.tensor_scalar_mul(pen[:], mask1[:], 1.0e30)
            lm = work.tile([128, NB, E], F32, tag="lm")
            nc.vector.tensor_sub(lm[:], l[:], pen[:])
            m2 = work.tile([128, NB], F32, tag="m2")
            nc.vector.reduce_max(m2[:], lm[:], AX.X)
            mask2 = work.tile([128, NB, E], F32, tag="mask2")
            nc.vector.tensor_tensor(
                mask2[:], lm[:], m2[:, :, None].to_broadcast([128, NB, E]),
                ALU.is_equal,
            )
            d = work.tile([128, NB], F32, tag="d")
            nc.vector.tensor_sub(d[:], m1[:], m2[:])
            w1 = work.tile([128, NB], F32, tag="w1")
            nc.scalar.activation(w1[:], d[:], AF.Sigmoid)
            w2 = work.tile([128, NB], F32, tag="w2")
            nc.vector.tensor_scalar(w2[:], w1[:], -1.0, 1.0, ALU.mult, ALU.add)
            t1 = work.tile([128, NB, E], F32, tag="t1")
            nc.vector.tensor_tensor(
                t1[:], mask1[:], w1[:, :, None].to_broadcast([128, NB, E]),
                ALU.mult,
            )
            t2 = work.tile([128, NB, E], F32, tag="t2")
            nc.vector.tensor_tensor(
                t2[:], mask2[:], w2[:, :, None].to_broadcast([128, NB, E]),
                ALU.mult,
            )
            wdt = work.tile([128, NB, E], F32, tag="wdt")
            nc.vector.tensor_add(wdt[:], t1[:], t2[:])
            nc.sync.dma_start(
                out=wd.rearrange("(b p) e -> p b e", p=128), in_=wdt[:]
            )
    return nc


def build_expert(cap: int = CAP) -> bass.Bass:
    """Per-core SwiGLU for one expert over CAP gathered tokens (bf16 matmuls,
    fp32 accumulate):  y^T = w * (silu(xe @ W1) * (xe @ W3)) @ W2, xe = x + dom.

    Inputs:  xt   [128, HS, CAP]    bf16  (xt[p, s, c]  = x_sel[c, s*128+p])
             w13t [128, IS, 2, HS, 128] bf16 (w13t[p,i,0,s,k] = W1[s*128+p, i*128+k];
                                        w13t[p,i,1,s,k] = W3[...])
             w2t  [128, IS, HS, 128] bf16 (w2t[p, j, t, k] = W2[j*128+p, t*128+k])
             dom  [128, HS]          bf16 (dom[p, s] = dom_e[s*128+p])
             wrep [128, CAP]         f32  (combine weight, replicated over partitions)
    Output:  yt   [H, CAP] f32  (yt[h, c] = y_sel[c, h])
    """
    nc = bass.Bass()
    xt = nc.dram_tensor("xt", [128, HS, cap], BF16, kind="ExternalInput")
    w13t = nc.dram_tensor("w13t", [128, IS, 2, HS, 128], BF16, kind="ExternalInput")
    w2t = nc.dram_tensor("w2t", [128, IS, HS, 128], BF16, kind="ExternalInput")
    dom = nc.dram_tensor("dom", [128, HS], BF16, kind="ExternalInput")
    wrep = nc.dram_tensor("wrep", [128, cap], F32, kind="ExternalInput")
    yt = nc.dram_tensor("yt", [H, cap], F32, kind="ExternalOutput")

    with _TileContext(nc) as tc:
        with (
            tc.tile_pool(name="const", bufs=1) as const,
            tc.tile_pool(name="wstream", bufs=6) as wstream,
            tc.tile_pool(name="hpool", bufs=1) as hpool,
            tc.tile_pool(name="work", bufs=4) as work,
            tc.tile_pool(name="ps_ab", bufs=4, space="PSUM") as ps_ab,
        ):
            # startup-critical DMAs first: xe feeds the very first matmuls
            dom_sb = const.tile([128, HS], BF16, tag="dom")
            nc.sync.dma_start(out=dom_sb[:], in_=dom[:])

            # PE warm-up: ~5us of garbage matmuls during the input DMA so the
            # HAM clock gate reaches 2.4 GHz before the real stream begins.
            wu = const.tile([128, 512], BF16, tag="warmup")
            nc.vector.memset(wu[:], 0)
            wu_ps = ps_ab.tile([128, 512], F32, tag="pa")
            for i in range(20):
                nc.tensor.matmul(
                    wu_ps[:],
                    lhsT=wu[:, :128],
                    rhs=wu[:],
                    start=(i == 0),
                    stop=(i == 19),
                )
            # DMA transfers drain in dispatch order, so interleave the first
            # I-tiles' W1/W3 slabs with the first xe sub-tiles: the PE gets
            # work as soon as each (slab, xe chunk) pair lands.
            NI = 4  # I-tile groups interleaved s-major during the xe fill
            pre_slabs = []
            xe_s = []
            for s in range(HS):
                if s < NI:
                    w13_sb = wstream.tile([128, 2, HS, 128], BF16, tag="w13")
                    nc.sync.dma_start(out=w13_sb[:], in_=w13t[:, s, :, :, :])
                    pre_slabs.append(w13_sb)
                xe = const.tile([128, cap], BF16, tag=f"xe{s}")
                nc.sync.dma_start(out=xe[:], in_=xt[:, s, :])
                nc.vector.tensor_tensor(
                    xe[:],
                    xe[:],
                    dom_sb[:, s : s + 1].to_broadcast([128, cap]),
                    ALU.add,
                )
                xe_s.append(xe)
            # wrep and W2 are first needed by phase 2 (~140us in); emitted
            # later (inside the first tile's loop) to keep them off the
            # startup-critical DMA window.
            wr_sb = const.tile([128, cap], F32, tag="wrep")
            w2_sb = const.tile([128, IS, HS, 128], BF16, tag="w2")

            def phase1_group(pa, pb, it, t0, tt, h_sb):
                sa = work.tile([128, 512], F32, tag="sa")
                nc.scalar.activation(sa[:, :tt], pa[:, :tt], AF.Silu)
                nc.vector.tensor_tensor(
                    h_sb[:, it, :tt], sa[:, :tt], pb[:, :tt], ALU.mult
                )

            for tile_idx, (t0, tt) in enumerate(_t_tiles(cap)):
                h_sb = hpool.tile([128, IS, 512], BF16, tag="h")
                # phase 1: a = xe @ W1, b = xe @ W3, h = silu(a) * b
                if tile_idx == 0:
                    # s-major across NI open PSUM groups: consume each xe
                    # sub-tile as its DMA lands instead of stalling on the
                    # full transfer.
                    pas, pbs = [], []
                    for k in range(NI):
                        pa = ps_ab.tile([128, 512], F32, tag="pa", name=f"pa0_{k}")
                        pb = ps_ab.tile([128, 512], F32, tag="pb", name=f"pb0_{k}")
                        pas.append(pa)
                        pbs.append(pb)
                    for s in range(HS):
                        for k in range(NI):
                            nc.tensor.matmul(
                                pas[k][:, :tt],
                                lhsT=pre_slabs[k][:, 0, s, :],
                                rhs=xe_s[s][:, t0 : t0 + tt],
                                start=(s == 0),
                                stop=(s == HS - 1),
                            )
                            nc.tensor.matmul(
                                pbs[k][:, :tt],
                                lhsT=pre_slabs[k][:, 1, s, :],
                                rhs=xe_s[s][:, t0 : t0 + tt],
                                start=(s == 0),
                                stop=(s == HS - 1),
                            )
                    for k in range(NI):
                        phase1_group(pas[k], pbs[k], k, t0, tt, h_sb)
                for it in range(NI if tile_idx == 0 else 0, IS):
                    w13_sb = wstream.tile([128, 2, HS, 128], BF16, tag="w13")
                    nc.sync.dma_start(out=w13_sb[:], in_=w13t[:, it, :, :, :])
                    if tile_idx == 0:
                        # wrep/W2 first used by phase 2; emit past the
                        # slot-recycle point so their transfers stay out of
                        # the xe fill window.
                        if it == 2 * NI:
                            nc.sync.dma_start(out=wr_sb[:], in_=wrep[:])
                        if it >= 2 * NI:
                            nc.sync.dma_start(
                                out=w2_sb[:, it, :, :], in_=w2t[:, it, :, :]
                            )
                        if it == IS - 1:
                            for j in range(2 * NI):
                                nc.sync.dma_start(
                                    out=w2_sb[:, j, :, :], in_=w2t[:, j, :, :]
                                )
                    pa = ps_ab.tile([128, 512], F32, tag="pa")
                    pb = ps_ab.tile([128, 512], F32, tag="pb")
                    for s in range(HS):
                        nc.tensor.matmul(
                            pa[:, :tt],
                            lhsT=w13_sb[:, 0, s, :],
                            rhs=xe_s[s][:, t0 : t0 + tt],
                            start=(s == 0),
                            stop=(s == HS - 1),
                        )
                    for s in range(HS):
                        nc.tensor.matmul(
                            pb[:, :tt],
                            lhsT=w13_sb[:, 1, s, :],
                            rhs=xe_s[s][:, t0 : t0 + tt],
                            start=(s == 0),
                            stop=(s == HS - 1),
                        )
                    phase1_group(pa, pb, it, t0, tt, h_sb)
                # phase 2: y^T tile = w * (h @ W2)^T
                for ht in range(HS):
                    py = ps_ab.tile([128, 512], F32, tag="pa", name=f"py_{tile_idx}_{ht}")
                    for j in range(IS):
                        nc.tensor.matmul(
                            py[:, :tt],
                            lhsT=w2_sb[:, j, ht, :],
                            rhs=h_sb[:, j, :tt],
                            start=(j == 0),
                            stop=(j == IS - 1),
                        )
                    yo = work.tile([128, 512], F32, tag="yo")
                    nc.vector.tensor_tensor(
                        yo[:, :tt], py[:, :tt], wr_sb[:, t0 : t0 + tt], ALU.mult
                    )
                    nc.sync.dma_start(
                        out=yt[ht * 128 : (ht + 1) * 128, t0 : t0 + tt],
                        in_=yo[:, :tt],
                    )
    return nc


_PROGRAMS: dict = {}


def _get_program(name, cap=CAP):
    key = (name, cap)
    if key not in _PROGRAMS:
        _PROGRAMS[key] = build_router() if name == "router" else build_expert(cap)
    return _PROGRAMS[key]


def _hs_split(a):
    """[D0, ...] with D0 = s*128+p  ->  [128, HS, ...] with [p, s, ...]."""
    return np.ascontiguousarray(
        a.reshape(HS, 128, *a.shape[1:]).swapaxes(0, 1)
    )


def kernel(hidden_states, gate_w, W1, W2, W3, dom):
    B, S, Hd = hidden_states.shape
    x2d = np.ascontiguousarray(
        np.asarray(hidden_states, dtype=np.float32).reshape(-1, Hd)
    )
    gate_w = np.asarray(gate_w, dtype=np.float32)
    W1 = np.asarray(W1, dtype=np.float32)
    W2 = np.asarray(W2, dtype=np.float32)
    W3 = np.asarray(W3, dtype=np.float32)
    dom = np.asarray(dom, dtype=np.float32)

    # ---- launch 1: router -------------------------------------------------
    gw_host = _hs_split(gate_w)  # [128, HS, E]
    in_maps1 = []
    for c in range(8):
        xs = x2d[c * TPC : (c + 1) * TPC]              # [TPC, H]
        xt = _hs_split(np.ascontiguousarray(xs.T))      # [128, HS, TPC]
        in_maps1.append({"xt": xt, "gw": gw_host})
    res1 = run_bass_kernel_spmd(_get_program("router"), in_maps1, list(range(8)))
    wd = np.concatenate([res1.results[c]["wd"] for c in range(8)], axis=0)  # [T, E]

    # ---- host dispatch ----------------------------------------------------
    x_bf = x2d.astype(NP_BF16)
    idxs = [np.nonzero(wd[:, e])[0] for e in range(E)]
    nsel = [len(idx) for idx in idxs]
    # fixed capacity normally; rebuild wider (multiple of 128) if ever exceeded
    cap = max(CAP, -(-max(nsel) // 128) * 128)
    in_maps2 = []
    for e in range(E):
        idx = idxs[e]
        n = nsel[e]
        pad_idx = np.zeros(cap, dtype=np.int64)
        pad_idx[:n] = idx
        w_sel = np.zeros(cap, dtype=np.float32)
        w_sel[:n] = wd[idx, e]

        xsel = x_bf[pad_idx]                            # [CAP, H]
        xt = _hs_split(np.ascontiguousarray(xsel.T))    # [128, HS, CAP] bf16
        w1r = W1[e].astype(NP_BF16).reshape(HS, 128, IS, 128).transpose(1, 2, 0, 3)
        w3r = W3[e].astype(NP_BF16).reshape(HS, 128, IS, 128).transpose(1, 2, 0, 3)
        w13t = np.ascontiguousarray(np.stack([w1r, w3r], axis=2))
        w2t = np.ascontiguousarray(
            W2[e].astype(NP_BF16).reshape(IS, 128, HS, 128).transpose(1, 0, 2, 3)
        )
        dom_t = np.ascontiguousarray(dom[e].astype(NP_BF16).reshape(HS, 128).T)
        wrep = np.ascontiguousarray(np.broadcast_to(w_sel, (128, cap)))
        in_maps2.append(
            {"xt": xt, "w13t": w13t, "w2t": w2t, "dom": dom_t, "wrep": wrep}
        )

    # ---- launch 2: experts ------------------------------------------------
    res2 = run_bass_kernel_spmd(_get_program("expert", cap), in_maps2, list(range(8)))

    # ---- host combine -----------------------------------------------------
    out = np.zeros((T, Hd), dtype=np.float32)
    for e in range(E):
        n = nsel[e]
        if n:
            yt = res2.results[e]["yt"]                  # [H, CAP] f32
            out[idxs[e]] += yt[:, :n].T
    return out.reshape(B, S, Hd)
